# revision 1
# baseline (speedup 1.0000x reference)
"""Distributed MHA kernel for one TRN2 chip (8 NeuronCores), Bass/Tile.

Problem: B=4, S=2048, D=1024, H=16 full multi-head attention
(qkv proj -> scaled dot product softmax attention -> o proj).

Sharding (no collectives): core c handles batch b=c//2 and query-token
half c%2 (1024 query tokens).  Each core recomputes K/V projections for
the full 2048 tokens of its batch (zero cross-core sync).  The host
permutes x[b] so the core's query tokens come first; softmax over keys
is permutation invariant, so K/V token order doesn't matter.

v2: all four projections (QKV and O) run as 3-term split-fp8
DoubleRow matmuls (W ~= Whi + Wlo, x ~= xhi + xlo in e4m3;
W@x ~= Whi@xhi + Whi@xlo + Wlo@xhi, residual error ~0.15%).
DoubleRow contracts 256 din per instruction at 0.5 cycles/row, so each
projection costs 0.75x its bf16 version.  The attention core stays
16-bit: fp8 logits/P/V fail the 2e-2 gate on sharp softmax rows where
elementwise quant noise does not average out.  Q/K are stored fp16
(e5m10) rather than bf16 — same matmul cost, 8x less quantization
noise on the few |logit|~100 rows.  P and V stay bf16 (P overflows
fp16's range), vals are rebuilt as e4m3 hi+lo pairs for the split O.

Emission interleaves everything: pair order alternates the two q512
chunks, and per-kc "filler" slots stream projection / o-proj pieces
(12 DoubleRow matmuls each) into the exp-paced attention stretches so
the PE (the critical engine at ~348us busy) almost never idles.  The
emission order also guarantees every tile's writer precedes its
readers in program order — a bias DMA emitted after its first reader
races on real hardware (reads of uninitialized SBUF carry no
dependency) and was the source of a nondeterministic corruption bug.

"""

import numpy as np

_NC_CACHE = {}


def _build_nc(S, D, H, SQ):
    import concourse.bass as bass
    import concourse.mybir as mybir
    import concourse.tile as tile
    from concourse import bacc
    from concourse.bass import ts

    f32 = mybir.dt.float32
    bf = mybir.dt.bfloat16
    f16 = mybir.dt.float16
    e4 = mybir.dt.float8e4
    Exp = mybir.ActivationFunctionType.Exp
    add = mybir.AluOpType.add
    mult = mybir.AluOpType.mult
    sub = mybir.AluOpType.subtract
    DR = mybir.MatmulPerfMode.DoubleRow

    P = 128
    hd = D // H            # 64 head dim
    hd1 = hd + 1           # 65: V block + ones column
    ND = D // P            # 8 dout chunks
    NC = D // 256          # 4 din DoubleRow pairs
    NT = S // 512          # 4 tok512 chunks (K/V)
    NQ = SQ // 512         # 2 q512 chunks
    NK = S // P            # 16 k-token chunks
    NG = D // 512          # 2 dv512 groups
    WS_INV = 1.0 / 32.0    # weights pre-scaled x32 for fp8; undone here
    scale = 1.0 / float(np.sqrt(hd))

    nc = bacc.Bacc(trn_type="TRN2", debug=False)

    # x and weights in DoubleRow din layout [p, pair, slot, cols]:
    # din d = 256*pair + 128*slot + p
    xhi = nc.declare_dram_parameter("xhi", [P, NC, 2, S], e4, isOutput=False)
    xlo = nc.declare_dram_parameter("xlo", [P, NC, 2, S], e4, isOutput=False)
    wts = {}
    for w in ("wq", "wk", "wv"):
        for part in ("hi", "lo"):
            name = f"{w}{part}"
            wts[name] = nc.declare_dram_parameter(
                name, [P, NC, 2, D], e4, isOutput=False)
    for part in ("hi", "lo"):
        wts[f"ow{part}"] = nc.declare_dram_parameter(
            f"ow{part}", [P, NC, 2, D], e4, isOutput=False)
    bq = nc.declare_dram_parameter("bq", [D], f32, isOutput=False)
    bk = nc.declare_dram_parameter("bk", [D], f32, isOutput=False)
    bv = nc.declare_dram_parameter("bv", [D], f32, isOutput=False)
    bo = nc.declare_dram_parameter("bo", [D], f32, isOutput=False)
    out = nc.declare_dram_parameter("out", [SQ, D], f32, isOutput=True)

    def mm(ps, lhsT, rhs, start, stop):
        nc.tensor.matmul(ps, lhsT, rhs, start=start, stop=stop)

    def mm8(ps, lhsT, rhs, start, stop):
        nc.tensor.matmul(ps, lhsT, rhs, start=start, stop=stop,
                         perf_mode=DR)

    with tile.TileContext(nc) as tc:
        with (
            tc.tile_pool(name="const", bufs=1) as constp,
            tc.tile_pool(name="xpool", bufs=1) as xpool,
            tc.tile_pool(name="kpool", bufs=1) as kpool,
            tc.tile_pool(name="vpool", bufs=1) as vpool,
            tc.tile_pool(name="qpool", bufs=1) as qpool,
            tc.tile_pool(name="wpool", bufs=4) as wpool,
            tc.tile_pool(name="wgpool", bufs=2) as wgpool,
            tc.tile_pool(name="valspool", bufs=2) as valspool,
            tc.tile_pool(name="ptpool", bufs=4) as ptpool,
            tc.tile_pool(name="opool", bufs=3) as opool,
            tc.tile_pool(name="lpool", bufs=2) as lpool,
            tc.tile_pool(name="lgps", bufs=2, space="PSUM") as lgps,
            tc.tile_pool(name="mmps", bufs=2, space="PSUM") as mmps,
            tc.tile_pool(name="pvps", bufs=2, space="PSUM") as pvps,
        ):
            # ---- x resident as fp8 hi/lo splits, staged so the first
            #      projection pieces start ~3us in ----
            xh = xpool.tile([P, NC, 2, S], e4, tag="xhi")
            xl = xpool.tile([P, NC, 2, S], e4, tag="xlo")

            # ---- constants: biases (DMAs deferred past first weights) ----
            bqs = constp.tile([P, ND], f32, tag="bq")
            bks = constp.tile([P, ND], f32, tag="bk")
            bvb = constp.tile([P, D], f32, tag="bv")
            bob = constp.tile([P, D], f32, tag="bo")

            def late_loads():
                nc.sync.dma_start(bvb[:],
                                  bv.ap().unsqueeze(0).to_broadcast((P, D)))
                nc.sync.dma_start(xh[:, :, :, SQ:S], xhi.ap()[:, :, :, SQ:S])
                nc.sync.dma_start(xl[:, :, :, SQ:S], xlo.ap()[:, :, :, SQ:S])
                nc.sync.dma_start(bob[:],
                                  bo.ap().unsqueeze(0).to_broadcast((P, D)))

            # ---- persistent K^T / V / Q^T in bf16 ----
            ksb = kpool.tile([P, ND, S], f16)          # K^T [2head, pair, tok]
            vsb = vpool.tile([P, NK, H, hd1], bf)     # V [tok_p, kc, head, 65]
            nc.vector.memset(vsb[:, :, :, hd:hd1], 0.125)  # 1/8 columns: vals x8
            qsb = qpool.tile([P, ND, SQ], f16)         # Q^T

            def proj_mm(ps, whi, wlo, xslice):
                """12 DoubleRow matmuls: Whi@xhi + Whi@xlo + Wlo@xhi."""
                first = True
                for wt, xt in ((whi, xh), (whi, xl), (wlo, xh)):
                    for c in range(NC):
                        mm8(ps[:], wt[:, c, :, :], xt[:, c, :, xslice],
                            first, (wt is wlo) and c == NC - 1)
                        first = False

            def _wload(wname, d, tag):
                nb = 2 if tag == "qw" else 4
                whi = wpool.tile([P, NC, 2, P], e4, tag=f"{tag}h",
                                 name=f"{tag}h{d}", bufs=nb)
                nc.sync.dma_start(whi[:], wts[f"{wname}hi"].ap()[:, :, :, ts(d, P)])
                wlo = wpool.tile([P, NC, 2, P], e4, tag=f"{tag}l",
                                 name=f"{tag}l{d}")
                nc.sync.dma_start(wlo[:], wts[f"{wname}lo"].ap()[:, :, :, ts(d, P)])
                return whi, wlo

            kw_cache = {}

            def q_piece(d, qi, w=None):
                # Q^T chunk d for q512 chunk qi (reloads weights per piece)
                whi, wlo = w if w is not None else _wload("wq", d, "qw")
                ps = mmps.tile([P, 512], f32, tag="mm")
                proj_mm(ps, whi, wlo, ts(qi, 512))
                nc.vector.tensor_scalar(qsb[:, d, ts(qi, 512)], ps[:],
                                        WS_INV, bqs[:, d:d + 1],
                                        mult, add)

            def k_piece(d, t):
                # K^T chunk d (heads 2d,2d+1), token block t
                if d not in kw_cache:
                    kw_cache[d] = _wload("wk", d, "kw")
                whi, wlo = kw_cache[d]
                ps = mmps.tile([P, 512], f32, tag="mm")
                proj_mm(ps, whi, wlo, ts(t, 512))
                nc.vector.tensor_scalar(ksb[:, d, ts(t, 512)], ps[:],
                                        WS_INV, bks[:, d:d + 1],
                                        mult, add)

            vw_cache = {}
            ow_cache = {}

            def v_piece(g, kc):
                # V dv-group g (heads 8g..8g+7), k-token chunk kc
                if g not in vw_cache:
                    pair = []
                    for part in ("hi", "lo"):
                        w = wgpool.tile([P, NC, 2, 512], e4, tag=f"vw{part}",
                                        name=f"vw{part}{g}")
                        nc.sync.dma_start(
                            w[:], wts[f"wv{part}"].ap()[:, :, :, ts(g, 512)])
                        pair.append(w)
                    vw_cache[g] = pair
                whi, wlo = vw_cache[g]
                ps = mmps.tile([P, 512], f32, tag="mm")
                first = True
                for wt, xt in ((whi, xh), (whi, xl), (wlo, xh)):
                    for c in range(NC):
                        mm8(ps[:], xt[:, c, :, ts(kc, P)], wt[:, c, :, :],
                            first, (wt is wlo) and c == NC - 1)
                        first = False
                dst = vsb[:, kc, ts(g, 512 // hd), 0:hd]
                nc.vector.scalar_tensor_tensor(
                    dst,
                    ps[:].rearrange("p (h e) -> p h e", e=hd),
                    WS_INV,
                    bvb[:, ts(g, 512)].rearrange("p (h e) -> p h e", e=hd),
                    op0=mult, op1=add)

            def o_piece(qi, g, si):
                # out rows [qi*512+si*128 ...], e-group g; 3-term DR
                if g not in ow_cache:
                    pair = []
                    for part in ("hi", "lo"):
                        w = wgpool.tile([P, NC, 2, 512], e4, tag=f"ow{part}",
                                        name=f"ow{part}{g}")
                        nc.sync.dma_start(
                            w[:], wts[f"ow{part}"].ap()[:, :, :, ts(g, 512)])
                        pair.append(w)
                    ow_cache[g] = pair
                owhi, owlo = ow_cache[g]
                ps = mmps.tile([P, 512], f32, tag="mm")
                first = True
                for c in range(NC):
                    for wt, part in ((owhi, 0), (owhi, 1), (owlo, 0)):
                        vt = valsbs[qi][c][part]
                        mm8(ps[:], vt[:, :, ts(si, P)], wt[:, c, :, :],
                            first, c == NC - 1 and wt is owlo)
                        first = False
                osb = opool.tile([P, 512], f32, tag="o")
                nc.vector.scalar_tensor_tensor(osb[:], ps[:], 1.0 / 256.0,
                                               bob[:, ts(g, 512)],
                                               op0=mult, op1=add)
                nc.sync.dma_start(
                    out.ap()[qi * 512 + si * P: qi * 512 + (si + 1) * P,
                             ts(g, 512)],
                    osb[:])

            def attn_pair(qi, p, fillers):
                # heads (2p, 2p+1) at partition offsets (0, 64); fillers
                # is a dict slot->emit_fn popped between exp and PV so the
                # PE never head-of-line blocks on the ACT-paced exp.
                vhi, vlo = valsbs[qi][p // 2]
                pd = p % 2
                pvs = [pvps.tile([hd1, 512], f32, tag="pv",
                                 name=f"pv{p}_{qi}_{j}") for j in range(2)]
                for kc in range(NK):
                    lg = lgps.tile([P, 2, 512], f32, tag="lg")
                    for j in range(2):
                        off = j * hd
                        mm(lg[:, j, :], ksb[off:off + hd, p, ts(kc, P)],
                           qsb[off:off + hd, p, ts(qi, 512)], True, True)
                    pt = ptpool.tile([P, 2, 512], bf, tag="pt")
                    nc.scalar.activation(pt[:], lg[:], Exp, scale=scale)
                    if kc in fillers:
                        for fn in fillers[kc]:
                            fn()
                    for j in range(2):
                        mm(pvs[j][:], vsb[:, kc, 2 * p + j, :], pt[:, j, :],
                           kc == 0, kc == NK - 1)
                pvc = []
                with tc.high_priority(offset=3000):
                    for j in range(2):
                        c = lpool.tile([hd1, 512], f32, tag="pvc")
                        nc.vector.tensor_copy(c[:], pvs[j][:])  # frees psum
                        pvc.append(c)
                for j in range(2):
                    off = j * hd
                    linv = lpool.tile([1, 512], f32, tag="linv", bufs=1)
                    nc.vector.reciprocal(linv[:], pvc[j][hd:hd1, :])
                    lbc = lpool.tile([hd, 512], f32, tag="lbc", bufs=1)
                    nc.gpsimd.partition_broadcast(lbc[:], linv[0:1, :])
                    vtmp = lpool.tile([P, 512], bf, tag="vtmp")
                    vts = vtmp[off:off + hd, :]
                    nc.vector.tensor_tensor(
                        vts, pvc[j][0:hd, :], lbc[:], op=mult)
                    nc.vector.tensor_copy(vhi[off:off + hd, pd, :], vts)
                    nc.vector.tensor_tensor(
                        vlo[off:off + hd, pd, :], vts,
                        vhi[off:off + hd, pd, :], op=sub)

            valsbs = []
            for qi in range(NQ):
                percs = []
                for c in range(NC):
                    vhi = valspool.tile([P, 2, 512], e4, tag=f"valshi{c}",
                                        name=f"valshi{qi}_{c}")
                    vlo = valspool.tile([P, 2, 512], e4, tag=f"valslo{c}",
                                        name=f"valslo{qi}_{c}")
                    percs.append((vhi, vlo))
                valsbs.append(percs)

            QP = lambda d, qi: (lambda: q_piece(d, qi))
            KP = lambda d, t: (lambda: k_piece(d, t))
            VP = lambda g, kc: (lambda: v_piece(g, kc))
            OP = lambda qi, g, si: (lambda: o_piece(qi, g, si))

            # ---- emission schedule ----
            # Pair order interleaves the two q512 chunks so projection and
            # o-proj pieces can fill every ACT-paced stretch.
            qw0 = _wload("wq", 0, "qw")
            kw_cache[0] = _wload("wk", 0, "kw")
            nc.sync.dma_start(bqs[:], bq.ap().rearrange("(c p) -> p c", p=P))
            nc.sync.dma_start(bks[:], bk.ap().rearrange("(c p) -> p c", p=P))
            nc.sync.dma_start(xh[:, :, :, 0:512], xhi.ap()[:, :, :, 0:512])
            nc.sync.dma_start(xl[:, :, :, 0:512], xlo.ap()[:, :, :, 0:512])
            nc.sync.dma_start(xh[:, :, :, 512:SQ], xhi.ap()[:, :, :, 512:SQ])
            nc.sync.dma_start(xl[:, :, :, 512:SQ], xlo.ap()[:, :, :, 512:SQ])
            q_piece(0, 0, w=qw0)
            k_piece(0, 0)
            k_piece(0, 1)
            late_loads()
            for kc in range(3):
                v_piece(0, kc)
            def ow_prefetch():
                for g in range(NG):
                    if g in ow_cache:
                        continue
                    pair = []
                    for part in ("hi", "lo"):
                        w = wgpool.tile([P, NC, 2, 512], e4, tag=f"ow{part}",
                                        name=f"ow{part}{g}")
                        nc.sync.dma_start(
                            w[:], wts[f"ow{part}"].ap()[:, :, :, ts(g, 512)])
                        pair.append(w)
                    ow_cache[g] = pair

            def slots(items):
                # spread items across the 16 kc slots, ~1 per slot
                return {kc: [it] for kc, it in enumerate(items)}

            sched = [
                # (qi, p, pre-list, filler items)
                (0, 0, [], [VP(0, 3), VP(0, 4), VP(0, 5), VP(0, 6), VP(0, 7),
                            VP(0, 8), VP(0, 9), KP(0, 2), VP(0, 10), VP(0, 11),
                            VP(0, 12), KP(0, 3), VP(0, 13), VP(0, 14),
                            VP(0, 15), QP(1, 0)]),
                (0, 1, [KP(1, 0)], [KP(1, 1), KP(1, 2), KP(1, 3), QP(2, 0),
                                    VP(1, 0), VP(1, 1), VP(1, 2), VP(1, 3)]),
                (0, 2, [KP(2, 0)], [KP(2, 1), KP(2, 2), KP(2, 3), QP(3, 0),
                                    VP(1, 4), VP(1, 5), VP(1, 6), VP(1, 7)]),
                (0, 3, [KP(3, 0)], [KP(3, 1), KP(3, 2), KP(3, 3), QP(0, 1),
                                    VP(1, 8), VP(1, 9), VP(1, 10), VP(1, 11)]),
                (1, 0, [ow_prefetch], [QP(1, 1), VP(1, 12), VP(1, 13),
                                       KP(4, 0)]),
                (1, 1, [], [QP(2, 1), KP(4, 1), KP(4, 2), KP(4, 3),
                            VP(1, 14)]),
                (1, 2, [], [QP(3, 1), KP(5, 0), KP(5, 1), VP(1, 15)]),
                (1, 3, [], [KP(5, 2), KP(5, 3), KP(6, 0), QP(4, 0)]),
                (0, 4, [], [KP(6, 1), KP(6, 2), QP(5, 0)]),
                (0, 5, [], [KP(6, 3), KP(7, 0), QP(6, 0)]),
                (0, 6, [], [KP(7, 1), KP(7, 2), KP(7, 3), QP(7, 0)]),
                (0, 7, [], [QP(4, 1), QP(5, 1)]),
                (1, 4, [], [OP(0, 0, 0), OP(0, 0, 1)]),
                (1, 5, [], [QP(6, 1), OP(0, 0, 2), OP(0, 0, 3)]),
                (1, 6, [], [QP(7, 1), OP(0, 1, 0), OP(0, 1, 1)]),
                (1, 7, [], [OP(0, 1, 2), OP(0, 1, 3)]),
            ]
            for qi, p, pre, items in sched:
                for fn in pre:
                    fn()
                attn_pair(qi, p, slots(items))
            for g in range(NG):
                for si in range(4):
                    o_piece(1, g, si)

    nc.compile()
    return nc


def _get_nc(S, D, H, SQ, use_bf16=True):
    key = (S, D, H, SQ)
    if key not in _NC_CACHE:
        _NC_CACHE[key] = _build_nc(S, D, H, SQ)
    return _NC_CACHE[key]


def _split8(arr):
    import ml_dtypes
    e4 = ml_dtypes.float8_e4m3
    hi = arr.astype(e4)
    lo = (arr - hi.astype(np.float32)).astype(e4)
    return np.ascontiguousarray(hi), np.ascontiguousarray(lo)


def _dr_layout(wT, P=128):
    """[din, cols] -> [p, pair, slot, cols] with din = 256*pair+128*slot+p."""
    D2, cols = wT.shape
    return np.ascontiguousarray(
        wT.reshape(D2 // 256, 2, P, cols).transpose(2, 0, 1, 3))


def _host_prep_weights(qkv_w, qkv_b, o_w, o_b, H):
    """Head-major q/k/v blocks, pre-transposed, fp8 hi/lo splits (x32)."""
    import ml_dtypes
    D = o_w.shape[0]
    hd = D // H
    qkv3 = qkv_w.reshape(H, 3, hd, D)
    b3 = qkv_b.reshape(H, 3, hd)
    m = {}
    for i, wname in enumerate(("wq", "wk", "wv")):
        wT = np.ascontiguousarray(qkv3[:, i].reshape(D, D).T) * 32.0
        hi, lo = _split8(_dr_layout(wT))
        m[f"{wname}hi"] = hi
        m[f"{wname}lo"] = lo
    owT = np.ascontiguousarray(o_w.T) * 32.0
    hi, lo = _split8(_dr_layout(owT))
    m["owhi"] = hi
    m["owlo"] = lo
    m["bq"] = np.ascontiguousarray(b3[:, 0].reshape(D))
    m["bk"] = np.ascontiguousarray(b3[:, 1].reshape(D))
    m["bv"] = np.ascontiguousarray(b3[:, 2].reshape(D))
    m["bo"] = np.ascontiguousarray(o_b)
    return m


def kernel(x, qkv_w, qkv_b, o_w, o_b, _trace=False):
    from concourse.bass_utils import run_bass_kernel_spmd

    x = np.asarray(x, dtype=np.float32)
    qkv_w = np.asarray(qkv_w, dtype=np.float32)
    qkv_b = np.asarray(qkv_b, dtype=np.float32)
    o_w = np.asarray(o_w, dtype=np.float32)
    o_b = np.asarray(o_b, dtype=np.float32)

    B, S, D = x.shape
    H = 16
    n_cores = 8
    halves = n_cores // B           # 2 query-token halves per batch
    SQ = S // halves                # 1024 query tokens per core

    nc = _get_nc(S, D, H, SQ)
    shared = _host_prep_weights(qkv_w, qkv_b, o_w, o_b, H)

    in_maps = []
    for c in range(n_cores):
        b, half = divmod(c, halves)
        # this core's query tokens first; key/value order is irrelevant
        xp = np.concatenate([x[b, half * SQ:(half + 1) * SQ],
                             np.concatenate([x[b, :half * SQ],
                                             x[b, (half + 1) * SQ:]], axis=0)],
                            axis=0)
        hi, lo = _split8(_dr_layout(np.ascontiguousarray(xp.T)))
        m = dict(shared)
        m["xhi"] = hi
        m["xlo"] = lo
        in_maps.append(m)

    res = run_bass_kernel_spmd(nc, in_maps, list(range(n_cores)),
                               trace=_trace)

    out = np.empty((B, S, D), dtype=np.float32)
    for c in range(n_cores):
        b, half = divmod(c, halves)
        out[b, half * SQ:(half + 1) * SQ] = res.results[c]["out"]
    if _trace:
        return out, res
    return out



# revision 44
# speedup vs baseline: 1.1219x; 1.1219x over previous
"""Distributed MHA kernel for one TRN2 chip (8 NeuronCores), Bass/Tile.

Problem: B=4, S=2048, D=1024, H=16 full multi-head attention
(qkv proj -> scaled dot product softmax attention -> o proj).

Sharding (no collectives): core c handles batch b=c//2 and query-token
half c%2 (1024 query tokens).  Each core recomputes K/V projections for
the full 2048 tokens of its batch (zero cross-core sync).  The host
permutes x[b] so the core's query tokens come first; softmax over keys
is permutation invariant, so K/V token order doesn't matter.

v2: all four projections (QKV and O) run as 3-term split-fp8
DoubleRow matmuls (W ~= Whi + Wlo, x ~= xhi + xlo in e4m3;
W@x ~= Whi@xhi + Whi@xlo + Wlo@xhi, residual error ~0.15%).
DoubleRow contracts 256 din per instruction at 0.5 cycles/row, so each
projection costs 0.75x its bf16 version.  The attention core stays
16-bit: fp8 logits/P/V fail the 2e-2 gate on sharp softmax rows where
elementwise quant noise does not average out.  Q/K are stored fp16
(e5m10) rather than bf16 — same matmul cost, 8x less quantization
noise on the few |logit|~100 rows.  P and V stay bf16 (P overflows
fp16's range), vals are rebuilt as e4m3 hi+lo pairs for the split O.

Emission interleaves everything: pair order alternates the two q512
chunks, and per-kc "filler" slots stream projection / o-proj pieces
(12 DoubleRow matmuls each) into the exp-paced attention stretches so
the PE (the critical engine at ~348us busy) almost never idles.  The
emission order also guarantees every tile's writer precedes its
readers in program order — a bias DMA emitted after its first reader
races on real hardware (reads of uninitialized SBUF carry no
dependency) and was the source of a nondeterministic corruption bug.

"""

import numpy as np

_NC_CACHE = {}


def _build_nc(S, D, H, SQ):
    import concourse.bass as bass
    import concourse.mybir as mybir
    import concourse.tile as tile
    from concourse import bacc
    from concourse.bass import ts

    f32 = mybir.dt.float32
    bf = mybir.dt.bfloat16
    f16 = mybir.dt.float16
    e4 = mybir.dt.float8e4
    Exp = mybir.ActivationFunctionType.Exp
    add = mybir.AluOpType.add
    mult = mybir.AluOpType.mult
    sub = mybir.AluOpType.subtract
    DR = mybir.MatmulPerfMode.DoubleRow

    P = 128
    hd = D // H            # 64 head dim
    ND = D // P            # 8 dout chunks
    NC = D // 256          # 4 din DoubleRow pairs
    NT = S // 512          # 4 tok512 chunks (K/V)
    NQ = SQ // 512         # 2 q512 chunks
    NK = S // P            # 16 k-token chunks
    NG = D // 512          # 2 dv512 groups
    WS_INV = 1.0 / 32.0    # weights pre-scaled x32 for fp8; undone here
    scale = 1.0 / float(np.sqrt(hd))

    nc = bacc.Bacc(trn_type="TRN2", debug=False)

    # x and weights in DoubleRow din layout [p, pair, slot, cols]:
    # din d = 256*pair + 128*slot + p
    xhi = nc.declare_dram_parameter("xhi", [P, NC, 2, S], e4, isOutput=False)
    xlo = nc.declare_dram_parameter("xlo", [P, NC, 2, S], e4, isOutput=False)
    wts = {}
    for w in ("wq", "wk", "wv"):
        for part in ("hi", "lo"):
            name = f"{w}{part}"
            wts[name] = nc.declare_dram_parameter(
                name, [P, NC, 2, D], e4, isOutput=False)
    for part in ("hi", "lo"):
        wts[f"ow{part}"] = nc.declare_dram_parameter(
            f"ow{part}", [P, NC, 2, D], e4, isOutput=False)
    bq = nc.declare_dram_parameter("bq", [D], f32, isOutput=False)
    bk = nc.declare_dram_parameter("bk", [D], f32, isOutput=False)
    bv = nc.declare_dram_parameter("bv", [D], f32, isOutput=False)
    bo = nc.declare_dram_parameter("bo", [D], f32, isOutput=False)
    # bf16 output halves the out-DMA traffic; host upcasts to f32
    out = nc.declare_dram_parameter("out", [SQ, D], bf, isOutput=True)

    def mm(ps, lhsT, rhs, start, stop):
        nc.tensor.matmul(ps, lhsT, rhs, start=start, stop=stop)

    def mm8(ps, lhsT, rhs, start, stop):
        nc.tensor.matmul(ps, lhsT, rhs, start=start, stop=stop,
                         perf_mode=DR)

    with tile.TileContext(nc) as tc:
        with (
            tc.tile_pool(name="const", bufs=1) as constp,
            tc.tile_pool(name="xpool", bufs=1) as xpool,
            tc.tile_pool(name="kpool", bufs=1) as kpool,
            tc.tile_pool(name="vpool", bufs=1) as vpool,
            tc.tile_pool(name="qpool", bufs=1) as qpool,
            tc.tile_pool(name="wpool", bufs=4) as wpool,
            tc.tile_pool(name="wgpool", bufs=2) as wgpool,
            tc.tile_pool(name="valspool", bufs=2) as valspool,
            tc.tile_pool(name="ptpool", bufs=4) as ptpool,
            tc.tile_pool(name="opool", bufs=3) as opool,
            tc.tile_pool(name="lpool", bufs=2) as lpool,
            tc.tile_pool(name="lgps", bufs=2, space="PSUM") as lgps,
            tc.tile_pool(name="mmps", bufs=1, space="PSUM") as mmps,
            tc.tile_pool(name="pvps", bufs=2, space="PSUM") as pvps,
            tc.tile_pool(name="dnps", bufs=1, space="PSUM") as dnps,
        ):
            # ---- x resident as fp8 hi/lo splits, staged so the first
            #      projection pieces start ~3us in ----
            xh = xpool.tile([P, NC, 2, S], e4, tag="xhi")
            xl = xpool.tile([P, NC, 2, S], e4, tag="xlo")

            # ---- constants: biases (DMAs deferred past first weights) ----
            bqs = constp.tile([P, ND], f32, tag="bq")
            bks = constp.tile([P, ND], f32, tag="bk")
            bvb = constp.tile([P, D], f32, tag="bv")
            bob = constp.tile([P, D], f32, tag="bo")



            # ---- persistent K^T / V / Q^T in bf16 ----
            ksb = kpool.tile([P, ND, S], f16)          # K^T [2head, pair, tok]
            vsb = vpool.tile([P, NK, H, hd], bf)      # V [tok_p, kc, head, e]
            qsb = qpool.tile([P, ND, SQ], f16)         # Q^T
            # ones column (x1/8) for the flipped-PV denominator matmuls
            onesb = constp.tile([P, 1], bf, tag="ones")
            nc.vector.memset(onesb[:], 0.125)

            def proj_mm(ps, whi, wlo, xslice, rng=range(12)):
                """12 DoubleRow matmuls: Whi@xhi + Whi@xlo + Wlo@xhi.
                rng selects a sub-range so a piece can be emitted in halves
                (6-matmul, ~640ns chunks) to pace PE work against the
                ACT-bound exp cadence."""
                seq = ((whi, xh), (whi, xl), (wlo, xh))
                for i in rng:
                    wt, xt = seq[i // NC]
                    mm8(ps[:], wt[:, i % NC, :, :], xt[:, i % NC, :, xslice],
                        i == 0, i == 11)

            def _wload(wname, d, tag):
                nb = 2 if tag == "qw" else 4
                whi = wpool.tile([P, NC, 2, P], e4, tag=f"{tag}h",
                                 name=f"{tag}h{d}", bufs=nb)
                nc.sync.dma_start(whi[:], wts[f"{wname}hi"].ap()[:, :, :, ts(d, P)])
                wlo = wpool.tile([P, NC, 2, P], e4, tag=f"{tag}l",
                                 name=f"{tag}l{d}")
                nc.sync.dma_start(wlo[:], wts[f"{wname}lo"].ap()[:, :, :, ts(d, P)])
                return whi, wlo

            kw_cache = {}

            _psrot = {"on": False, "alt": None, "i": 0}

            def _mm_ps(ps):
                # proj psum: default pool slot, or a caller-supplied region
                # (idle lg-pool banks during startup/tail, where back-to-back
                # pieces would otherwise stall on the single mm slot's drain).
                # While _psrot is on (pair 0: one full piece per slot),
                # alternate with a scratch region in the idle second pv-pool
                # slot so consecutive drains overlap.
                if ps is not None:
                    return ps
                if _psrot["on"]:
                    _psrot["i"] += 1
                    if _psrot["i"] % 2 == 0:
                        if _psrot["alt"] is None:
                            _psrot["alt"] = pvps.tile(
                                [P, 2, 4, hd], f32, tag="pv",
                                name="pvx")[:].rearrange(
                                    "p a b e -> p (a b e)")
                        return _psrot["alt"]
                return mmps.tile([P, 512], f32, tag="mm", name="ps")

            def q_piece(d, qi, w=None, ps=None, rng=range(12), st=None):
                # Q^T chunk d for q512 chunk qi (reloads weights per piece)
                if st is not None and "w" in st:
                    whi, wlo = st["w"]
                    ps = st["ps"]
                else:
                    whi, wlo = w if w is not None else _wload("wq", d, "qw")
                    ps = _mm_ps(ps)
                    if st is not None:
                        st["w"], st["ps"] = (whi, wlo), ps
                proj_mm(ps, whi, wlo, ts(qi, 512), rng)
                if rng[-1] == 11:
                    nc.vector.tensor_scalar(qsb[:, d, ts(qi, 512)], ps[:],
                                            WS_INV, bqs[:, d:d + 1],
                                            mult, add)

            def k_piece(d, t, ps=None, rng=range(12), st=None):
                # K^T chunk d (heads 2d,2d+1), token block t
                if st is not None and "w" in st:
                    whi, wlo = st["w"]
                    ps = st["ps"]
                else:
                    if d not in kw_cache:
                        kw_cache[d] = _wload("wk", d, "kw")
                    whi, wlo = kw_cache[d]
                    ps = _mm_ps(ps)
                    if st is not None:
                        st["w"], st["ps"] = (whi, wlo), ps
                proj_mm(ps, whi, wlo, ts(t, 512), rng)
                if rng[-1] == 11:
                    nc.vector.tensor_scalar(ksb[:, d, ts(t, 512)], ps[:],
                                            WS_INV, bks[:, d:d + 1],
                                            mult, add)

            vw_cache = {}
            ow_cache = {}

            def v_piece(g, kc, ps=None, rng=range(12), st=None):
                # V dv-group g (heads 8g..8g+7), k-token chunk kc
                if st is not None and "w" in st:
                    whi, wlo = st["w"]
                    ps = st["ps"]
                else:
                    if g not in vw_cache:
                        pair = []
                        for part in ("hi", "lo"):
                            w = wgpool.tile([P, NC, 2, 512], e4,
                                            tag=f"vw{part}",
                                            name=f"vw{part}{g}")
                            nc.sync.dma_start(
                                w[:],
                                wts[f"wv{part}"].ap()[:, :, :, ts(g, 512)])
                            pair.append(w)
                        vw_cache[g] = pair
                    whi, wlo = vw_cache[g]
                    ps = _mm_ps(ps)
                    if st is not None:
                        st["w"], st["ps"] = (whi, wlo), ps
                seq = ((whi, xh), (whi, xl), (wlo, xh))
                for i in rng:
                    wt, xt = seq[i // NC]
                    mm8(ps[:], xt[:, i % NC, :, ts(kc, P)], wt[:, i % NC, :, :],
                        i == 0, i == 11)
                if rng[-1] == 11:
                    dst = vsb[:, kc, ts(g, 512 // hd), 0:hd]
                    nc.vector.scalar_tensor_tensor(
                        dst,
                        ps[:].rearrange("p (h e) -> p h e", e=hd),
                        WS_INV,
                        bvb[:, ts(g, 512)].rearrange("p (h e) -> p h e", e=hd),
                        op0=mult, op1=add)

            def o_piece(qi, g, si, ps=None, rng=range(12), st=None):
                # out rows [qi*512+si*128 ...], e-group g; 3-term DR
                if st is not None and "w" in st:
                    owhi, owlo = st["w"]
                    ps = st["ps"]
                else:
                    if g not in ow_cache:
                        pair = []
                        for part in ("hi", "lo"):
                            w = wgpool.tile([P, NC, 2, 512], e4,
                                            tag=f"ow{part}",
                                            name=f"ow{part}{g}")
                            nc.sync.dma_start(
                                w[:],
                                wts[f"ow{part}"].ap()[:, :, :, ts(g, 512)])
                            pair.append(w)
                        ow_cache[g] = pair
                    owhi, owlo = ow_cache[g]
                    ps = _mm_ps(ps)
                    if st is not None:
                        st["w"], st["ps"] = (owhi, owlo), ps
                for i in rng:
                    c = i // 3
                    wt, part = ((owhi, 0), (owhi, 1), (owlo, 0))[i % 3]
                    vt = valsbs[qi][c][part]
                    mm8(ps[:], vt[:, :, ts(si, P)], wt[:, c, :, :],
                        i == 0, i == 11)
                if rng[-1] == 11:
                    osb = opool.tile([P, 512], bf, tag="o")
                    nc.vector.scalar_tensor_tensor(osb[:], ps[:], 1.0 / 256.0,
                                                   bob[:, ts(g, 512)],
                                                   op0=mult, op1=add)
                    nc.sync.dma_start(
                        out.ap()[qi * 512 + si * P: qi * 512 + (si + 1) * P,
                                 ts(g, 512)],
                        osb[:])

            # denominator psum; one accumulation group per pair, WAR against
            # the previous pair's reciprocal read orders reuse
            dn = dnps.tile([P, 2, 4, 1], f32, tag="dn")

            def attn_pair(qi, p, fillers):
                # heads (2p, 2p+1).  Flipped PV: stationary = pt q-block
                # [128k, 128q], streaming = V [128k, 64] -> out [128q, 64]
                # (half the streamed columns of the V-stationary form).
                # Denominators via ap-1 matmuls against a 0.125-ones column
                # reusing the same stationary.  Logits/exp run one kc ahead
                # of PV so the PE never waits on the ACT-paced exp.
                vhi, vlo = valsbs[qi][p // 2]
                pd = p % 2
                pv = pvps.tile([P, 2, 4, hd], f32, tag="pv",
                               name=f"pv{p}_{qi}")

                def emit_lg(kc):
                    lg = lgps.tile([P, 2, 512], f32, tag="lg")
                    for j in range(2):
                        off = j * hd
                        mm(lg[:, j, :], ksb[off:off + hd, p, ts(kc, P)],
                           qsb[off:off + hd, p, ts(qi, 512)], True, True)
                    pt = ptpool.tile([P, 2, 512], bf, tag="pt")
                    nc.scalar.activation(pt[:], lg[:], Exp, scale=scale)
                    return pt

                pts = {0: emit_lg(0)}
                for kc in range(NK):
                    if kc + 1 < NK:
                        pts[kc + 1] = emit_lg(kc + 1)
                    if kc in fillers:
                        for fn in fillers[kc]:
                            fn()
                    pt = pts.pop(kc)
                    for j in range(2):
                        for qs in range(4):
                            # ONE accumulation group per psum bank per pair:
                            # start=True zeroes the whole 2KB zero region, so
                            # only the first matmul into each tile starts and
                            # only the last stops.
                            first = kc == 0 and j == 0 and qs == 0
                            last = kc == NK - 1 and j == 1 and qs == 3
                            mm(pv[:, j, qs, :], pt[:, j, ts(qs, P)],
                               vsb[:, kc, 2 * p + j, :], first, last)
                            mm(dn[:, j, qs, :], pt[:, j, ts(qs, P)],
                               onesb[:], first, last)
                with tc.high_priority(offset=3000):
                    rc = lpool.tile([P, 2, 4, 1], f32, tag="rc")
                    nc.vector.reciprocal(rc[:], dn[:])  # rc = 8/L per q
                    vsc = lpool.tile([P, 4, 2, hd], bf, tag="vsc")
                    nc.vector.tensor_tensor(
                        vsc[:].rearrange("p a b e -> p b a e"), pv[:],
                        rc[:].to_broadcast((P, 2, 4, hd)), op=mult)
                # vals^T via DMA xbar transpose: out[p, c, q] = in[q, 128c+p]
                # with in free = (qs*128 + j*64 + e) -> out = [dv, qs, q128]
                vT = lpool.tile([P, 4, P], bf, tag="vT")
                nc.sync.dma_start_transpose(vT[:], vsc[:])
                nc.vector.tensor_copy(
                    vhi[:, pd, :].rearrange("p (a q) -> p a q", a=4), vT[:])
                nc.vector.tensor_tensor(
                    vlo[:, pd, :].rearrange("p (a q) -> p a q", a=4), vT[:],
                    vhi[:, pd, :].rearrange("p (a q) -> p a q", a=4), op=sub)

            valsbs = []
            for qi in range(NQ):
                percs = []
                for c in range(NC):
                    vhi = valspool.tile([P, 2, 512], e4, tag=f"valshi{c}",
                                        name=f"valshi{qi}_{c}")
                    vlo = valspool.tile([P, 2, 512], e4, tag=f"valslo{c}",
                                        name=f"valslo{qi}_{c}")
                    percs.append((vhi, vlo))
                valsbs.append(percs)

            QP = lambda d, qi: (lambda: q_piece(d, qi))
            KP = lambda d, t: (lambda: k_piece(d, t))
            VP = lambda g, kc: (lambda: v_piece(g, kc))
            OP = lambda qi, g, si: (lambda: o_piece(qi, g, si))

            def _halves(fn, *args):
                # split a 12-matmul piece into two ~640ns emissions (A then
                # B in a later slot) so filler PE work spreads evenly against
                # the ACT exp cadence.  A and B share psum/weights via st;
                # the two halves of a piece must not interleave with another
                # piece's halves (single mm psum slot).
                st = {}
                A = lambda: fn(*args, rng=range(6), st=st)
                B = lambda: fn(*args, rng=range(6, 12), st=st)
                return A, B

            QH = lambda d, qi: _halves(q_piece, d, qi)
            KH = lambda d, t: _halves(k_piece, d, t)
            VH = lambda g, kc: _halves(v_piece, g, kc)
            OH = lambda qi, g, si: _halves(o_piece, qi, g, si)

            def hsched(*placed):
                # placed: (slot, (A, B)) -> {slot: [A], slot+1: [B]}
                d = {}
                for s, (a, b) in placed:
                    d.setdefault(s, []).append(a)
                    d.setdefault(s + 1, []).append(b)
                return d

            # ---- emission schedule ----
            # Pair order interleaves the two q512 chunks so projection and
            # o-proj pieces can fill every ACT-paced stretch.
            # Startup DMAs ordered by first use: the DMA engines are an
            # exclusive resource in practice, so emission order is landing
            # order.  q piece needs qw + x[0:512]; k(0,0) needs kw; drains
            # need biases; v pieces need vw/bvb, then x[512:1024].
            qw0 = _wload("wq", 0, "qw")
            nc.scalar.dma_start(xh[:, :, :, 0:512], xhi.ap()[:, :, :, 0:512])
            nc.gpsimd.dma_start(xl[:, :, :, 0:512], xlo.ap()[:, :, :, 0:512])
            kw_cache[0] = _wload("wk", 0, "kw")
            nc.scalar.dma_start(xh[:, :, :, 512:SQ], xhi.ap()[:, :, :, 512:SQ])
            nc.gpsimd.dma_start(xl[:, :, :, 512:SQ], xlo.ap()[:, :, :, 512:SQ])
            nc.sync.dma_start(bqs[:], bq.ap().rearrange("(c p) -> p c", p=P))
            nc.sync.dma_start(bks[:], bk.ap().rearrange("(c p) -> p c", p=P))
            # V group-0 weights up front: pair 0's PV chain is V-piece-fed
            vw_pair = []
            for part, eng in (("hi", nc.scalar), ("lo", nc.gpsimd)):
                w = wgpool.tile([P, NC, 2, 512], e4, tag=f"vw{part}",
                                name=f"vw{part}0")
                eng.dma_start(
                    w[:], wts[f"wv{part}"].ap()[:, :, :, 0:512])
                vw_pair.append(w)
            vw_cache[0] = vw_pair
            nc.sync.dma_start(bvb[:],
                              bv.ap().unsqueeze(0).to_broadcast((P, D)))
            # startup pieces rotate through idle lg-pool banks so consecutive
            # drains never stall the PE on the single mm psum slot (WAR
            # tracking is tile-granular: each piece needs a distinct tile)
            lg_s0 = lgps.tile([P, 2, 512], f32, tag="lg", name="lgs0")
            lg_s1 = lgps.tile([P, 2, 512], f32, tag="lg", name="lgs1")
            q_piece(0, 0, w=qw0, ps=lg_s0[:, 0, :])
            k_piece(0, 0, ps=lg_s1[:, 0, :])
            v_piece(0, 0)
            v_piece(0, 1, ps=lg_s0[:, 1, :])
            # x tails + o-bias after the startup pieces' inputs
            nc.scalar.dma_start(xh[:, :, :, SQ:S], xhi.ap()[:, :, :, SQ:S])
            nc.gpsimd.dma_start(xl[:, :, :, SQ:S], xlo.ap()[:, :, :, SQ:S])
            nc.scalar.dma_start(bob[:],
                                bo.ap().unsqueeze(0).to_broadcast((P, D)))

            def ow_prefetch():
                for g in range(NG):
                    if g in ow_cache:
                        continue
                    pair = []
                    for part in ("hi", "lo"):
                        w = wgpool.tile([P, NC, 2, 512], e4, tag=f"ow{part}",
                                        name=f"ow{part}{g}")
                        nc.sync.dma_start(
                            w[:], wts[f"ow{part}"].ap()[:, :, :, ts(g, 512)])
                        pair.append(w)
                    ow_cache[g] = pair

            def slots(d):
                return {kc: (v if isinstance(v, list) else [v])
                        for kc, v in d.items()}

            # Fillers keyed by kc slot.  Deadlines under the 1-ahead pipe:
            # a KP(d,t) filler inside pair (0,d) must COMPLETE at slot
            # <= 4t-2 (lg[kc] is emitted at slot kc-1); VP(g,kc') complete
            # at slot <= kc' of its first consumer pair; QP anywhere before
            # the consuming pair starts.  Pair 0 self-feeds its V/K just in
            # time (full pieces); later pairs get half-pieces, ~1 per slot.
            sched = [
                # (qi, p, pre-list, {slot: fillers})
                (0, 0, [], {0: [VP(0, 2), KP(0, 1)], 1: VP(0, 3),
                            2: VP(0, 4), 3: VP(0, 5), 4: [VP(0, 6),
                            KP(0, 2)], 5: VP(0, 7), 6: VP(0, 8),
                            7: VP(0, 9), 8: [VP(0, 10), KP(0, 3)],
                            9: VP(0, 11), 10: VP(0, 12), 11: VP(0, 13),
                            12: VP(0, 14), 13: VP(0, 15),
                            14: [QP(1, 0)] + list(KH(1, 0))}),
                (0, 1, [],
                 hsched((0, KH(1, 1)), (3, KH(1, 2)), (6, KH(1, 3)),
                        (8, VH(1, 0)), (10, VH(1, 1)), (12, KH(2, 0)),
                        (14, QH(2, 0)))),
                (0, 2, [],
                 hsched((0, KH(2, 1)), (3, KH(2, 2)), (6, KH(2, 3)),
                        (8, VH(1, 2)), (10, VH(1, 3)), (12, KH(3, 0)),
                        (14, QH(3, 0)))),
                (0, 3, [],
                 hsched((0, KH(3, 1)), (3, KH(3, 2)), (6, KH(3, 3)),
                        (8, VH(1, 4)), (10, VH(1, 5)), (12, QH(0, 1)))),
                (1, 0, [ow_prefetch],
                 hsched((0, VH(1, 6)), (2, VH(1, 7)), (5, KH(4, 0)),
                        (8, KH(4, 1)), (11, QH(1, 1)), (13, VH(1, 8)))),
                (1, 1, [],
                 hsched((0, VH(1, 9)), (2, KH(4, 2)), (5, KH(4, 3)),
                        (8, VH(1, 10)), (11, QH(2, 1)), (13, KH(5, 0)))),
                (1, 2, [],
                 hsched((0, VH(1, 11)), (2, KH(5, 1)), (5, KH(5, 2)),
                        (8, VH(1, 12)), (11, QH(3, 1)), (13, KH(5, 3)))),
                (1, 3, [],
                 hsched((0, KH(6, 0)), (2, VH(1, 13)), (5, KH(6, 1)),
                        (8, QH(4, 0)), (11, KH(6, 2)), (13, VH(1, 14)))),
                (0, 4, [],
                 hsched((0, VH(1, 15)), (3, KH(6, 3)), (6, KH(7, 0)),
                        (9, QH(5, 0)), (12, KH(7, 1)))),
                (0, 5, [],
                 hsched((0, KH(7, 2)), (3, KH(7, 3)), (6, QH(6, 0)),
                        (9, QH(4, 1)), (12, QH(5, 1)))),
                (0, 6, [],
                 hsched((1, QH(7, 0)), (6, QH(6, 1)), (11, QH(7, 1)))),
                (0, 7, [], {}),
                (1, 4, [],
                 hsched((1, OH(0, 0, 0)), (6, OH(0, 0, 1)),
                        (11, OH(0, 0, 2)))),
                (1, 5, [],
                 hsched((1, OH(0, 0, 3)), (6, OH(0, 1, 0)),
                        (11, OH(0, 1, 1)))),
                (1, 6, [],
                 hsched((1, OH(0, 1, 2)), (6, OH(0, 1, 3)))),
                (1, 7, [], {}),
            ]
            for pos, (qi, p, pre, items) in enumerate(sched):
                for fn in pre:
                    fn()
                _psrot["on"] = pos == 0
                attn_pair(qi, p, slots(items))
            _psrot["on"] = False
            # tail o-proj pieces rotate five distinct psum tiles (the lg/pv
            # banks are idle once attention is done; WAR is tile-granular)
            lg_t0 = lgps.tile([P, 2, 512], f32, tag="lg", name="lgt0")
            lg_t1 = lgps.tile([P, 2, 512], f32, tag="lg", name="lgt1")
            pv_t0 = pvps.tile([P, 2, 4, hd], f32, tag="pv", name="pvt0")
            pv_t1 = pvps.tile([P, 2, 4, hd], f32, tag="pv", name="pvt1")
            tail_ps = [lg_t0[:, 0, :],
                       pv_t0[:].rearrange("p a b e -> p (a b e)"),
                       lg_t1[:, 0, :],
                       pv_t1[:].rearrange("p a b e -> p (a b e)"), None]
            i = 0
            for g in range(NG):
                for si in range(4):
                    o_piece(1, g, si, ps=tail_ps[i % 5])
                    i += 1

    nc.compile()
    return nc


def _get_nc(S, D, H, SQ, use_bf16=True):
    key = (S, D, H, SQ)
    if key not in _NC_CACHE:
        _NC_CACHE[key] = _build_nc(S, D, H, SQ)
    return _NC_CACHE[key]


def _split8(arr):
    import ml_dtypes
    e4 = ml_dtypes.float8_e4m3
    hi = arr.astype(e4)
    lo = (arr - hi.astype(np.float32)).astype(e4)
    return np.ascontiguousarray(hi), np.ascontiguousarray(lo)


def _dr_layout(wT, P=128):
    """[din, cols] -> [p, pair, slot, cols] with din = 256*pair+128*slot+p."""
    D2, cols = wT.shape
    return np.ascontiguousarray(
        wT.reshape(D2 // 256, 2, P, cols).transpose(2, 0, 1, 3))


def _host_prep_weights(qkv_w, qkv_b, o_w, o_b, H):
    """Head-major q/k/v blocks, pre-transposed, fp8 hi/lo splits (x32)."""
    import ml_dtypes
    D = o_w.shape[0]
    hd = D // H
    qkv3 = qkv_w.reshape(H, 3, hd, D)
    b3 = qkv_b.reshape(H, 3, hd)
    m = {}
    for i, wname in enumerate(("wq", "wk", "wv")):
        wT = np.ascontiguousarray(qkv3[:, i].reshape(D, D).T) * 32.0
        hi, lo = _split8(_dr_layout(wT))
        m[f"{wname}hi"] = hi
        m[f"{wname}lo"] = lo
    owT = np.ascontiguousarray(o_w.T) * 32.0
    hi, lo = _split8(_dr_layout(owT))
    m["owhi"] = hi
    m["owlo"] = lo
    m["bq"] = np.ascontiguousarray(b3[:, 0].reshape(D))
    m["bk"] = np.ascontiguousarray(b3[:, 1].reshape(D))
    m["bv"] = np.ascontiguousarray(b3[:, 2].reshape(D))
    m["bo"] = np.ascontiguousarray(o_b)
    return m


def kernel(x, qkv_w, qkv_b, o_w, o_b, _trace=False):
    from concourse.bass_utils import run_bass_kernel_spmd

    x = np.asarray(x, dtype=np.float32)
    qkv_w = np.asarray(qkv_w, dtype=np.float32)
    qkv_b = np.asarray(qkv_b, dtype=np.float32)
    o_w = np.asarray(o_w, dtype=np.float32)
    o_b = np.asarray(o_b, dtype=np.float32)

    B, S, D = x.shape
    H = 16
    n_cores = 8
    halves = n_cores // B           # 2 query-token halves per batch
    SQ = S // halves                # 1024 query tokens per core

    nc = _get_nc(S, D, H, SQ)
    shared = _host_prep_weights(qkv_w, qkv_b, o_w, o_b, H)

    in_maps = []
    for c in range(n_cores):
        b, half = divmod(c, halves)
        # this core's query tokens first; key/value order is irrelevant
        xp = np.concatenate([x[b, half * SQ:(half + 1) * SQ],
                             np.concatenate([x[b, :half * SQ],
                                             x[b, (half + 1) * SQ:]], axis=0)],
                            axis=0)
        hi, lo = _split8(_dr_layout(np.ascontiguousarray(xp.T)))
        m = dict(shared)
        m["xhi"] = hi
        m["xlo"] = lo
        in_maps.append(m)

    res = run_bass_kernel_spmd(nc, in_maps, list(range(n_cores)),
                               trace=_trace)

    out = np.empty((B, S, D), dtype=np.float32)
    for c in range(n_cores):
        b, half = divmod(c, halves)
        out[b, half * SQ:(half + 1) * SQ] = res.results[c]["out"]
    if _trace:
        return out, res
    return out



# revision 45
# speedup vs baseline: 1.1293x; 1.0066x over previous
"""Distributed MHA kernel for one TRN2 chip (8 NeuronCores), Bass/Tile.

Problem: B=4, S=2048, D=1024, H=16 full multi-head attention
(qkv proj -> scaled dot product softmax attention -> o proj).

Sharding (no collectives): core c handles batch b=c//2 and query-token
half c%2 (1024 query tokens).  Each core recomputes K/V projections for
the full 2048 tokens of its batch (zero cross-core sync).  The host
permutes x[b] so the core's query tokens come first; softmax over keys
is permutation invariant, so K/V token order doesn't matter.

v2: all four projections (QKV and O) run as 3-term split-fp8
DoubleRow matmuls (W ~= Whi + Wlo, x ~= xhi + xlo in e4m3;
W@x ~= Whi@xhi + Whi@xlo + Wlo@xhi, residual error ~0.15%).
DoubleRow contracts 256 din per instruction at 0.5 cycles/row, so each
projection costs 0.75x its bf16 version.  The attention core stays
16-bit: fp8 logits/P/V fail the 2e-2 gate on sharp softmax rows where
elementwise quant noise does not average out.  Q/K are stored fp16
(e5m10) rather than bf16 — same matmul cost, 8x less quantization
noise on the few |logit|~100 rows.  P and V stay bf16 (P overflows
fp16's range), vals are rebuilt as e4m3 hi+lo pairs for the split O.

Emission interleaves everything: pair order alternates the two q512
chunks, and per-kc "filler" slots stream projection / o-proj pieces
(12 DoubleRow matmuls each) into the exp-paced attention stretches so
the PE (the critical engine at ~348us busy) almost never idles.  The
emission order also guarantees every tile's writer precedes its
readers in program order — a bias DMA emitted after its first reader
races on real hardware (reads of uninitialized SBUF carry no
dependency) and was the source of a nondeterministic corruption bug.

"""

import numpy as np

_NC_CACHE = {}


def _build_nc(S, D, H, SQ):
    import concourse.bass as bass
    import concourse.mybir as mybir
    import concourse.tile as tile
    from concourse import bacc
    from concourse.bass import ts

    f32 = mybir.dt.float32
    bf = mybir.dt.bfloat16
    f16 = mybir.dt.float16
    e4 = mybir.dt.float8e4
    Exp = mybir.ActivationFunctionType.Exp
    add = mybir.AluOpType.add
    mult = mybir.AluOpType.mult
    sub = mybir.AluOpType.subtract
    DR = mybir.MatmulPerfMode.DoubleRow

    P = 128
    hd = D // H            # 64 head dim
    ND = D // P            # 8 dout chunks
    NC = D // 256          # 4 din DoubleRow pairs
    NT = S // 512          # 4 tok512 chunks (K/V)
    NQ = SQ // 512         # 2 q512 chunks
    NK = S // P            # 16 k-token chunks
    NG = D // 512          # 2 dv512 groups
    WS_INV = 1.0 / 32.0    # weights pre-scaled x32 for fp8; undone here
    scale = 1.0 / float(np.sqrt(hd))

    nc = bacc.Bacc(trn_type="TRN2", debug=False)

    # x and weights in DoubleRow din layout [p, pair, slot, cols]:
    # din d = 256*pair + 128*slot + p
    xhi = nc.declare_dram_parameter("xhi", [P, NC, 2, S], e4, isOutput=False)
    xlo = nc.declare_dram_parameter("xlo", [P, NC, 2, S], e4, isOutput=False)
    wts = {}
    for w in ("wq", "wk", "wv"):
        for part in ("hi", "lo"):
            name = f"{w}{part}"
            wts[name] = nc.declare_dram_parameter(
                name, [P, NC, 2, D], e4, isOutput=False)
    for part in ("hi", "lo"):
        wts[f"ow{part}"] = nc.declare_dram_parameter(
            f"ow{part}", [P, NC, 2, D], e4, isOutput=False)
    bq = nc.declare_dram_parameter("bq", [D], f32, isOutput=False)
    bk = nc.declare_dram_parameter("bk", [D], f32, isOutput=False)
    bv = nc.declare_dram_parameter("bv", [D], f32, isOutput=False)
    bo = nc.declare_dram_parameter("bo", [D], f32, isOutput=False)
    # bf16 output halves the out-DMA traffic; host upcasts to f32
    out = nc.declare_dram_parameter("out", [SQ, D], bf, isOutput=True)

    def mm(ps, lhsT, rhs, start, stop):
        nc.tensor.matmul(ps, lhsT, rhs, start=start, stop=stop)

    def mm8(ps, lhsT, rhs, start, stop):
        nc.tensor.matmul(ps, lhsT, rhs, start=start, stop=stop,
                         perf_mode=DR)

    with tile.TileContext(nc) as tc:
        with (
            tc.tile_pool(name="const", bufs=1) as constp,
            tc.tile_pool(name="xpool", bufs=1) as xpool,
            tc.tile_pool(name="kpool", bufs=1) as kpool,
            tc.tile_pool(name="vpool", bufs=1) as vpool,
            tc.tile_pool(name="qpool", bufs=1) as qpool,
            tc.tile_pool(name="wpool", bufs=4) as wpool,
            tc.tile_pool(name="wgpool", bufs=2) as wgpool,
            tc.tile_pool(name="valspool", bufs=2) as valspool,
            tc.tile_pool(name="ptpool", bufs=4) as ptpool,
            tc.tile_pool(name="opool", bufs=3) as opool,
            tc.tile_pool(name="lpool", bufs=2) as lpool,
            tc.tile_pool(name="lgps", bufs=2, space="PSUM") as lgps,
            tc.tile_pool(name="mmps", bufs=1, space="PSUM") as mmps,
            tc.tile_pool(name="pvps", bufs=2, space="PSUM") as pvps,
            tc.tile_pool(name="dnps", bufs=1, space="PSUM") as dnps,
        ):
            # ---- x resident as fp8 hi/lo splits, staged so the first
            #      projection pieces start ~3us in ----
            xh = xpool.tile([P, NC, 2, S], e4, tag="xhi")
            xl = xpool.tile([P, NC, 2, S], e4, tag="xlo")

            # ---- constants: biases (DMAs deferred past first weights) ----
            bqs = constp.tile([P, ND], f32, tag="bq")
            bks = constp.tile([P, ND], f32, tag="bk")
            bvb = constp.tile([P, D], f32, tag="bv")
            bob = constp.tile([P, D], f32, tag="bo")



            # ---- persistent K^T / V / Q^T in bf16 ----
            ksb = kpool.tile([P, ND, S], f16)          # K^T [2head, pair, tok]
            vsb = vpool.tile([P, NK, H, hd], bf)      # V [tok_p, kc, head, e]
            qsb = qpool.tile([P, ND, SQ], f16)         # Q^T
            # ones column (x1/8) for the flipped-PV denominator matmuls
            onesb = constp.tile([P, 1], bf, tag="ones")
            nc.vector.memset(onesb[:], 0.125)

            def proj_mm(ps, whi, wlo, xslice, rng=range(12)):
                """12 DoubleRow matmuls: Whi@xhi + Whi@xlo + Wlo@xhi.
                rng selects a sub-range so a piece can be emitted in halves
                (6-matmul, ~640ns chunks) to pace PE work against the
                ACT-bound exp cadence."""
                seq = ((whi, xh), (whi, xl), (wlo, xh))
                for i in rng:
                    wt, xt = seq[i // NC]
                    mm8(ps[:], wt[:, i % NC, :, :], xt[:, i % NC, :, xslice],
                        i == 0, i == 11)

            def _wload(wname, d, tag):
                nb = 2 if tag == "qw" else 4
                whi = wpool.tile([P, NC, 2, P], e4, tag=f"{tag}h",
                                 name=f"{tag}h{d}", bufs=nb)
                nc.sync.dma_start(whi[:], wts[f"{wname}hi"].ap()[:, :, :, ts(d, P)])
                wlo = wpool.tile([P, NC, 2, P], e4, tag=f"{tag}l",
                                 name=f"{tag}l{d}")
                nc.sync.dma_start(wlo[:], wts[f"{wname}lo"].ap()[:, :, :, ts(d, P)])
                return whi, wlo

            kw_cache = {}

            _psrot = {"on": False, "alt": None, "i": 0}

            def _mm_ps(ps):
                # proj psum: default pool slot, or a caller-supplied region
                # (idle lg-pool banks during startup/tail, where back-to-back
                # pieces would otherwise stall on the single mm slot's drain).
                # While _psrot is on (pair 0: one full piece per slot),
                # alternate with a scratch region in the idle second pv-pool
                # slot so consecutive drains overlap.
                if ps is not None:
                    return ps
                if _psrot["on"]:
                    _psrot["i"] += 1
                    if _psrot["i"] % 2 == 0:
                        if _psrot["alt"] is None:
                            _psrot["alt"] = pvps.tile(
                                [P, 2, 4, hd], f32, tag="pv",
                                name="pvx")[:].rearrange(
                                    "p a b e -> p (a b e)")
                        return _psrot["alt"]
                return mmps.tile([P, 512], f32, tag="mm", name="ps")

            def q_piece(d, qi, w=None, ps=None, rng=range(12), st=None):
                # Q^T chunk d for q512 chunk qi (reloads weights per piece)
                if st is not None and "w" in st:
                    whi, wlo = st["w"]
                    ps = st["ps"]
                else:
                    whi, wlo = w if w is not None else _wload("wq", d, "qw")
                    ps = _mm_ps(ps)
                    if st is not None:
                        st["w"], st["ps"] = (whi, wlo), ps
                proj_mm(ps, whi, wlo, ts(qi, 512), rng)
                if rng[-1] == 11:
                    nc.vector.tensor_scalar(qsb[:, d, ts(qi, 512)], ps[:],
                                            WS_INV, bqs[:, d:d + 1],
                                            mult, add)

            def k_piece(d, t, ps=None, rng=range(12), st=None):
                # K^T chunk d (heads 2d,2d+1), token block t
                if st is not None and "w" in st:
                    whi, wlo = st["w"]
                    ps = st["ps"]
                else:
                    if d not in kw_cache:
                        kw_cache[d] = _wload("wk", d, "kw")
                    whi, wlo = kw_cache[d]
                    ps = _mm_ps(ps)
                    if st is not None:
                        st["w"], st["ps"] = (whi, wlo), ps
                proj_mm(ps, whi, wlo, ts(t, 512), rng)
                if rng[-1] == 11:
                    nc.vector.tensor_scalar(ksb[:, d, ts(t, 512)], ps[:],
                                            WS_INV, bks[:, d:d + 1],
                                            mult, add)

            vw_cache = {}
            ow_cache = {}

            def v_piece(g, kc, ps=None, rng=range(12), st=None):
                # V dv-group g (heads 8g..8g+7), k-token chunk kc
                if st is not None and "w" in st:
                    whi, wlo = st["w"]
                    ps = st["ps"]
                else:
                    if g not in vw_cache:
                        pair = []
                        for part in ("hi", "lo"):
                            w = wgpool.tile([P, NC, 2, 512], e4,
                                            tag=f"vw{part}",
                                            name=f"vw{part}{g}")
                            nc.sync.dma_start(
                                w[:],
                                wts[f"wv{part}"].ap()[:, :, :, ts(g, 512)])
                            pair.append(w)
                        vw_cache[g] = pair
                    whi, wlo = vw_cache[g]
                    ps = _mm_ps(ps)
                    if st is not None:
                        st["w"], st["ps"] = (whi, wlo), ps
                seq = ((whi, xh), (whi, xl), (wlo, xh))
                for i in rng:
                    wt, xt = seq[i // NC]
                    mm8(ps[:], xt[:, i % NC, :, ts(kc, P)], wt[:, i % NC, :, :],
                        i == 0, i == 11)
                if rng[-1] == 11:
                    dst = vsb[:, kc, ts(g, 512 // hd), 0:hd]
                    nc.vector.scalar_tensor_tensor(
                        dst,
                        ps[:].rearrange("p (h e) -> p h e", e=hd),
                        WS_INV,
                        bvb[:, ts(g, 512)].rearrange("p (h e) -> p h e", e=hd),
                        op0=mult, op1=add)

            def o_piece(qi, g, si, ps=None, rng=range(12), st=None):
                # out rows [qi*512+si*128 ...], e-group g; 3-term DR
                if st is not None and "w" in st:
                    owhi, owlo = st["w"]
                    ps = st["ps"]
                else:
                    if g not in ow_cache:
                        pair = []
                        for part in ("hi", "lo"):
                            w = wgpool.tile([P, NC, 2, 512], e4,
                                            tag=f"ow{part}",
                                            name=f"ow{part}{g}")
                            nc.sync.dma_start(
                                w[:],
                                wts[f"ow{part}"].ap()[:, :, :, ts(g, 512)])
                            pair.append(w)
                        ow_cache[g] = pair
                    owhi, owlo = ow_cache[g]
                    ps = _mm_ps(ps)
                    if st is not None:
                        st["w"], st["ps"] = (owhi, owlo), ps
                for i in rng:
                    c = i // 3
                    wt, part = ((owhi, 0), (owhi, 1), (owlo, 0))[i % 3]
                    vt = valsbs[qi][c][part]
                    mm8(ps[:], vt[:, :, ts(si, P)], wt[:, c, :, :],
                        i == 0, i == 11)
                if rng[-1] == 11:
                    osb = opool.tile([P, 512], bf, tag="o")
                    nc.vector.scalar_tensor_tensor(osb[:], ps[:], 1.0 / 256.0,
                                                   bob[:, ts(g, 512)],
                                                   op0=mult, op1=add)
                    nc.sync.dma_start(
                        out.ap()[qi * 512 + si * P: qi * 512 + (si + 1) * P,
                                 ts(g, 512)],
                        osb[:])

            # denominator psum; one accumulation group per pair, WAR against
            # the previous pair's reciprocal read orders reuse
            dn = dnps.tile([P, 2, 4, 1], f32, tag="dn")

            def attn_pair(qi, p, fillers):
                # heads (2p, 2p+1).  Flipped PV: stationary = pt q-block
                # [128k, 128q], streaming = V [128k, 64] -> out [128q, 64]
                # (half the streamed columns of the V-stationary form).
                # Denominators via ap-1 matmuls against a 0.125-ones column
                # reusing the same stationary.  Logits/exp run one kc ahead
                # of PV so the PE never waits on the ACT-paced exp.
                vhi, vlo = valsbs[qi][p // 2]
                pd = p % 2
                pv = pvps.tile([P, 2, 4, hd], f32, tag="pv",
                               name=f"pv{p}_{qi}")

                def emit_lg(kc):
                    lg = lgps.tile([P, 2, 512], f32, tag="lg")
                    for j in range(2):
                        off = j * hd
                        mm(lg[:, j, :], ksb[off:off + hd, p, ts(kc, P)],
                           qsb[off:off + hd, p, ts(qi, 512)], True, True)
                    pt = ptpool.tile([P, 2, 512], bf, tag="pt")
                    nc.scalar.activation(pt[:], lg[:], Exp, scale=scale)
                    return pt

                pts = {0: emit_lg(0)}
                for kc in range(NK):
                    if kc + 1 < NK:
                        pts[kc + 1] = emit_lg(kc + 1)
                    if kc in fillers:
                        for fn in fillers[kc]:
                            fn()
                    pt = pts.pop(kc)
                    for j in range(2):
                        for qs in range(4):
                            # ONE accumulation group per psum bank per pair:
                            # start=True zeroes the whole 2KB zero region, so
                            # only the first matmul into each tile starts and
                            # only the last stops.
                            first = kc == 0 and j == 0 and qs == 0
                            last = kc == NK - 1 and j == 1 and qs == 3
                            mm(pv[:, j, qs, :], pt[:, j, ts(qs, P)],
                               vsb[:, kc, 2 * p + j, :], first, last)
                            mm(dn[:, j, qs, :], pt[:, j, ts(qs, P)],
                               onesb[:], first, last)
                with tc.high_priority(offset=3000):
                    rc = lpool.tile([P, 2, 4, 1], f32, tag="rc")
                    nc.vector.reciprocal(rc[:], dn[:])  # rc = 8/L per q
                    vsc = lpool.tile([P, 4, 2, hd], bf, tag="vsc")
                    nc.vector.tensor_tensor(
                        vsc[:].rearrange("p a b e -> p b a e"), pv[:],
                        rc[:].to_broadcast((P, 2, 4, hd)), op=mult)
                # vals^T via DMA xbar transpose: out[p, c, q] = in[q, 128c+p]
                # with in free = (qs*128 + j*64 + e) -> out = [dv, qs, q128]
                vT = lpool.tile([P, 4, P], bf, tag="vT")
                nc.sync.dma_start_transpose(vT[:], vsc[:])
                nc.vector.tensor_copy(
                    vhi[:, pd, :].rearrange("p (a q) -> p a q", a=4), vT[:])
                nc.vector.tensor_tensor(
                    vlo[:, pd, :].rearrange("p (a q) -> p a q", a=4), vT[:],
                    vhi[:, pd, :].rearrange("p (a q) -> p a q", a=4), op=sub)

            valsbs = []
            for qi in range(NQ):
                percs = []
                for c in range(NC):
                    vhi = valspool.tile([P, 2, 512], e4, tag=f"valshi{c}",
                                        name=f"valshi{qi}_{c}")
                    vlo = valspool.tile([P, 2, 512], e4, tag=f"valslo{c}",
                                        name=f"valslo{qi}_{c}")
                    percs.append((vhi, vlo))
                valsbs.append(percs)

            QP = lambda d, qi: (lambda: q_piece(d, qi))
            KP = lambda d, t: (lambda: k_piece(d, t))
            VP = lambda g, kc: (lambda: v_piece(g, kc))
            OP = lambda qi, g, si: (lambda: o_piece(qi, g, si))

            def _halves(fn, *args):
                # split a 12-matmul piece into two ~640ns emissions (A then
                # B in a later slot) so filler PE work spreads evenly against
                # the ACT exp cadence.  A and B share psum/weights via st;
                # the two halves of a piece must not interleave with another
                # piece's halves (single mm psum slot).
                st = {}
                A = lambda: fn(*args, rng=range(6), st=st)
                B = lambda: fn(*args, rng=range(6, 12), st=st)
                return A, B

            QH = lambda d, qi: _halves(q_piece, d, qi)
            KH = lambda d, t: _halves(k_piece, d, t)
            VH = lambda g, kc: _halves(v_piece, g, kc)
            OH = lambda qi, g, si: _halves(o_piece, qi, g, si)

            def hsched(*placed):
                # placed: (slot, (A, B)) -> {slot: [A], slot+1: [B]}
                d = {}
                for s, (a, b) in placed:
                    d.setdefault(s, []).append(a)
                    d.setdefault(s + 1, []).append(b)
                return d

            # ---- emission schedule ----
            # Pair order interleaves the two q512 chunks so projection and
            # o-proj pieces can fill every ACT-paced stretch.
            # Startup DMAs ordered by first use: the DMA engines are an
            # exclusive resource in practice, so emission order is landing
            # order.  q piece needs qw + x[0:512]; k(0,0) needs kw; drains
            # need biases; v pieces need vw/bvb, then x[512:1024].
            qw0 = _wload("wq", 0, "qw")
            nc.scalar.dma_start(xh[:, :, :, 0:512], xhi.ap()[:, :, :, 0:512])
            nc.gpsimd.dma_start(xl[:, :, :, 0:512], xlo.ap()[:, :, :, 0:512])
            kw_cache[0] = _wload("wk", 0, "kw")
            nc.sync.dma_start(bqs[:], bq.ap().rearrange("(c p) -> p c", p=P))
            nc.sync.dma_start(bks[:], bk.ap().rearrange("(c p) -> p c", p=P))
            nc.scalar.dma_start(xh[:, :, :, 512:SQ], xhi.ap()[:, :, :, 512:SQ])
            nc.gpsimd.dma_start(xl[:, :, :, 512:SQ], xlo.ap()[:, :, :, 512:SQ])
            # V group-0 weights up front: pair 0's PV chain is V-piece-fed
            vw_pair = []
            for part, eng in (("hi", nc.scalar), ("lo", nc.gpsimd)):
                w = wgpool.tile([P, NC, 2, 512], e4, tag=f"vw{part}",
                                name=f"vw{part}0")
                eng.dma_start(
                    w[:], wts[f"wv{part}"].ap()[:, :, :, 0:512])
                vw_pair.append(w)
            vw_cache[0] = vw_pair
            nc.sync.dma_start(bvb[:],
                              bv.ap().unsqueeze(0).to_broadcast((P, D)))
            # x tails ride the sync queue BEHIND the critical weights/biases:
            # queue order is landing order at the (serial) DMA engines, and
            # the tails are not needed until pair-0 slot 8.
            nc.sync.dma_start(xh[:, :, :, SQ:S], xhi.ap()[:, :, :, SQ:S])
            nc.sync.dma_start(xl[:, :, :, SQ:S], xlo.ap()[:, :, :, SQ:S])
            nc.sync.dma_start(bob[:],
                              bo.ap().unsqueeze(0).to_broadcast((P, D)))
            # startup pieces rotate through idle lg-pool banks so consecutive
            # drains never stall the PE on the single mm psum slot (WAR
            # tracking is tile-granular: each piece needs a distinct tile)
            lg_s0 = lgps.tile([P, 2, 512], f32, tag="lg", name="lgs0")
            lg_s1 = lgps.tile([P, 2, 512], f32, tag="lg", name="lgs1")
            q_piece(0, 0, w=qw0, ps=lg_s0[:, 0, :])
            k_piece(0, 0, ps=lg_s1[:, 0, :])
            v_piece(0, 0)
            v_piece(0, 1, ps=lg_s0[:, 1, :])

            def ow_prefetch():
                for g in range(NG):
                    if g in ow_cache:
                        continue
                    pair = []
                    for part in ("hi", "lo"):
                        w = wgpool.tile([P, NC, 2, 512], e4, tag=f"ow{part}",
                                        name=f"ow{part}{g}")
                        nc.sync.dma_start(
                            w[:], wts[f"ow{part}"].ap()[:, :, :, ts(g, 512)])
                        pair.append(w)
                    ow_cache[g] = pair

            def slots(d):
                return {kc: (v if isinstance(v, list) else [v])
                        for kc, v in d.items()}

            # Fillers keyed by kc slot.  Deadlines under the 1-ahead pipe:
            # a KP(d,t) filler inside pair (0,d) must COMPLETE at slot
            # <= 4t-2 (lg[kc] is emitted at slot kc-1); VP(g,kc') complete
            # at slot <= kc' of its first consumer pair; QP anywhere before
            # the consuming pair starts.  Pair 0 self-feeds its V/K just in
            # time (full pieces); later pairs get half-pieces, ~1 per slot.
            sched = [
                # (qi, p, pre-list, {slot: fillers})
                (0, 0, [], {0: [VP(0, 2), KP(0, 1)], 1: VP(0, 3),
                            2: VP(0, 4), 3: VP(0, 5), 4: [VP(0, 6),
                            KP(0, 2)], 5: VP(0, 7), 6: VP(0, 8),
                            7: VP(0, 9), 8: [VP(0, 10), KP(0, 3)],
                            9: VP(0, 11), 10: VP(0, 12), 11: VP(0, 13),
                            12: VP(0, 14), 13: VP(0, 15),
                            14: [QP(1, 0)] + list(KH(1, 0))}),
                (0, 1, [],
                 hsched((0, KH(1, 1)), (3, KH(1, 2)), (6, KH(1, 3)),
                        (8, VH(1, 0)), (10, VH(1, 1)), (12, KH(2, 0)),
                        (14, QH(2, 0)))),
                (0, 2, [],
                 hsched((0, KH(2, 1)), (3, KH(2, 2)), (6, KH(2, 3)),
                        (8, VH(1, 2)), (10, VH(1, 3)), (12, KH(3, 0)),
                        (14, QH(3, 0)))),
                (0, 3, [],
                 hsched((0, KH(3, 1)), (3, KH(3, 2)), (6, KH(3, 3)),
                        (8, VH(1, 4)), (10, VH(1, 5)), (12, QH(0, 1)))),
                (1, 0, [ow_prefetch],
                 hsched((0, VH(1, 6)), (2, VH(1, 7)), (5, KH(4, 0)),
                        (8, KH(4, 1)), (11, QH(1, 1)), (13, VH(1, 8)))),
                (1, 1, [],
                 hsched((0, VH(1, 9)), (2, KH(4, 2)), (5, KH(4, 3)),
                        (8, VH(1, 10)), (11, QH(2, 1)), (13, KH(5, 0)))),
                (1, 2, [],
                 hsched((0, VH(1, 11)), (2, KH(5, 1)), (5, KH(5, 2)),
                        (8, VH(1, 12)), (11, QH(3, 1)), (13, KH(5, 3)))),
                (1, 3, [],
                 hsched((0, KH(6, 0)), (2, VH(1, 13)), (5, KH(6, 1)),
                        (8, QH(4, 0)), (11, KH(6, 2)), (13, VH(1, 14)))),
                (0, 4, [],
                 hsched((0, VH(1, 15)), (3, KH(6, 3)), (6, KH(7, 0)),
                        (9, QH(5, 0)), (12, KH(7, 1)))),
                (0, 5, [],
                 hsched((0, KH(7, 2)), (3, KH(7, 3)), (6, QH(6, 0)),
                        (9, QH(4, 1)), (12, QH(5, 1)))),
                (0, 6, [],
                 hsched((1, QH(7, 0)), (6, QH(6, 1)), (11, QH(7, 1)))),
                (0, 7, [], {}),
                (1, 4, [],
                 hsched((1, OH(0, 0, 0)), (6, OH(0, 0, 1)),
                        (11, OH(0, 0, 2)))),
                (1, 5, [],
                 hsched((1, OH(0, 0, 3)), (6, OH(0, 1, 0)),
                        (11, OH(0, 1, 1)))),
                (1, 6, [],
                 hsched((1, OH(0, 1, 2)), (6, OH(0, 1, 3)))),
                (1, 7, [], {}),
            ]
            for pos, (qi, p, pre, items) in enumerate(sched):
                for fn in pre:
                    fn()
                _psrot["on"] = pos == 0
                attn_pair(qi, p, slots(items))
            _psrot["on"] = False
            # tail o-proj pieces rotate five distinct psum tiles (the lg/pv
            # banks are idle once attention is done; WAR is tile-granular)
            lg_t0 = lgps.tile([P, 2, 512], f32, tag="lg", name="lgt0")
            lg_t1 = lgps.tile([P, 2, 512], f32, tag="lg", name="lgt1")
            pv_t0 = pvps.tile([P, 2, 4, hd], f32, tag="pv", name="pvt0")
            pv_t1 = pvps.tile([P, 2, 4, hd], f32, tag="pv", name="pvt1")
            tail_ps = [lg_t0[:, 0, :],
                       pv_t0[:].rearrange("p a b e -> p (a b e)"),
                       lg_t1[:, 0, :],
                       pv_t1[:].rearrange("p a b e -> p (a b e)"), None]
            i = 0
            for g in range(NG):
                for si in range(4):
                    o_piece(1, g, si, ps=tail_ps[i % 5])
                    i += 1

    nc.compile()
    return nc


def _get_nc(S, D, H, SQ, use_bf16=True):
    key = (S, D, H, SQ)
    if key not in _NC_CACHE:
        _NC_CACHE[key] = _build_nc(S, D, H, SQ)
    return _NC_CACHE[key]


def _split8(arr):
    import ml_dtypes
    e4 = ml_dtypes.float8_e4m3
    hi = arr.astype(e4)
    lo = (arr - hi.astype(np.float32)).astype(e4)
    return np.ascontiguousarray(hi), np.ascontiguousarray(lo)


def _dr_layout(wT, P=128):
    """[din, cols] -> [p, pair, slot, cols] with din = 256*pair+128*slot+p."""
    D2, cols = wT.shape
    return np.ascontiguousarray(
        wT.reshape(D2 // 256, 2, P, cols).transpose(2, 0, 1, 3))


def _host_prep_weights(qkv_w, qkv_b, o_w, o_b, H):
    """Head-major q/k/v blocks, pre-transposed, fp8 hi/lo splits (x32)."""
    import ml_dtypes
    D = o_w.shape[0]
    hd = D // H
    qkv3 = qkv_w.reshape(H, 3, hd, D)
    b3 = qkv_b.reshape(H, 3, hd)
    m = {}
    for i, wname in enumerate(("wq", "wk", "wv")):
        wT = np.ascontiguousarray(qkv3[:, i].reshape(D, D).T) * 32.0
        hi, lo = _split8(_dr_layout(wT))
        m[f"{wname}hi"] = hi
        m[f"{wname}lo"] = lo
    owT = np.ascontiguousarray(o_w.T) * 32.0
    hi, lo = _split8(_dr_layout(owT))
    m["owhi"] = hi
    m["owlo"] = lo
    m["bq"] = np.ascontiguousarray(b3[:, 0].reshape(D))
    m["bk"] = np.ascontiguousarray(b3[:, 1].reshape(D))
    m["bv"] = np.ascontiguousarray(b3[:, 2].reshape(D))
    m["bo"] = np.ascontiguousarray(o_b)
    return m


def kernel(x, qkv_w, qkv_b, o_w, o_b, _trace=False):
    from concourse.bass_utils import run_bass_kernel_spmd

    x = np.asarray(x, dtype=np.float32)
    qkv_w = np.asarray(qkv_w, dtype=np.float32)
    qkv_b = np.asarray(qkv_b, dtype=np.float32)
    o_w = np.asarray(o_w, dtype=np.float32)
    o_b = np.asarray(o_b, dtype=np.float32)

    B, S, D = x.shape
    H = 16
    n_cores = 8
    halves = n_cores // B           # 2 query-token halves per batch
    SQ = S // halves                # 1024 query tokens per core

    nc = _get_nc(S, D, H, SQ)
    shared = _host_prep_weights(qkv_w, qkv_b, o_w, o_b, H)

    in_maps = []
    for c in range(n_cores):
        b, half = divmod(c, halves)
        # this core's query tokens first; key/value order is irrelevant
        xp = np.concatenate([x[b, half * SQ:(half + 1) * SQ],
                             np.concatenate([x[b, :half * SQ],
                                             x[b, (half + 1) * SQ:]], axis=0)],
                            axis=0)
        hi, lo = _split8(_dr_layout(np.ascontiguousarray(xp.T)))
        m = dict(shared)
        m["xhi"] = hi
        m["xlo"] = lo
        in_maps.append(m)

    res = run_bass_kernel_spmd(nc, in_maps, list(range(n_cores)),
                               trace=_trace)

    out = np.empty((B, S, D), dtype=np.float32)
    for c in range(n_cores):
        b, half = divmod(c, halves)
        out[b, half * SQ:(half + 1) * SQ] = res.results[c]["out"]
    if _trace:
        return out, res
    return out

